# revision 26
# baseline (speedup 1.0000x reference)
"""ASSR reconstruction loss on 8 Trainium2 NeuronCores.

total = mean|pred-target| + 0.1 * (mean|bicubic_aa_resize(pred, 128,128) - lr_ref|)

Data-parallel over the batch axis: core i handles batches [4i, 4i+4).
Each core returns two partial sums (pix L1 sum, lr L1 sum); the host
combines them into the 5 reference outputs.
"""

import numpy as np
from contextlib import ExitStack

import concourse.bass as bass
import concourse.bacc as bacc
import concourse.tile as tile
import concourse.mybir as mybir
from concourse.bass_utils import run_bass_kernel_spmd

F32 = mybir.dt.float32
AF = mybir.ActivationFunctionType
ALU = mybir.AluOpType
AX = mybir.AxisListType

N_CORES = 8
B, C, H, W = 32, 3, 512, 512
TH, TW = 128, 128
BS = B // N_CORES         # batches per core
IMGS = BS * C             # images per core
HC = H // 128             # h chunks of 128
WC = W // 128             # w chunks of 128

LAM_CONSIST = 0.1
LAM_LR = 1.0

# build options; tuned via TimelineSim + HW experiments
DEFAULT_OPTS = dict(
    batch_dma=False,     # load per-batch [3MiB] tiles instead of per-image [1MiB]
    split_rings=False,   # issue targ loads from the ACT HWDGE ring
    bufs=4,              # pred/targ pool depth (per-image mode)
    inplace_abs=False,   # ACT abs writes over the diff tile
    compute=True,        # False = DMA-only skeleton (for floor measurement)
    reps=1,              # replicate the main loop (for HW timing deltas)
    timing_dram=False,   # image tensors are Internal DRAM scratch (no host xfer)
    flat_img=True,       # per-partition-contiguous image DMA layout
    lr_swdge=False,      # issue lr_ref loads from the gpsimd SWDGE queue
)


def _cubic(x, a=-0.75):
    # float32, mirrors the reference's PyTorch bicubic kernel
    ax = np.abs(x)
    ax2 = ax * ax
    ax3 = ax2 * ax
    f1 = (a + 2.0) * ax3 - (a + 3.0) * ax2 + 1.0
    f2 = a * ax3 - 5.0 * a * ax2 + 8.0 * a * ax - 4.0 * a
    return np.where(ax <= 1.0, f1, np.where(ax < 2.0, f2, np.float32(0.0)))


def _resize_matrix(in_size: int, out_size: int) -> np.ndarray:
    scale = in_size / out_size
    s_aa = max(scale, 1.0)
    support = 2.0 * s_aa
    ext = int(np.ceil(support)) + 1
    centers = (np.arange(out_size, dtype=np.float64) + 0.5) * scale - 0.5
    idx = np.arange(-ext, in_size + ext)
    dist = (idx[None, :] - centers[:, None]) / s_aa
    w = _cubic(dist.astype(np.float32)).astype(np.float32)
    w = w / np.sum(w, axis=1, keepdims=True)
    idx_c = np.clip(idx, 0, in_size - 1)
    M = np.zeros((out_size, in_size), dtype=np.float32)
    np.add.at(M, (np.arange(out_size)[:, None], idx_c[None, :].repeat(out_size, 0)), w)
    return M


_CACHE = {}


def _build(**opts):
    o = {**DEFAULT_OPTS, **opts}
    assert not (o["batch_dma"] and o["flat_img"])
    key = tuple(sorted(o.items()))
    if key in _CACHE:
        return _CACHE[key]

    nc = bacc.Bacc("TRN2", target_bir_lowering=False, debug=False,
                   num_devices=N_CORES)
    img_kind = "Internal" if o["timing_dram"] else "ExternalInput"
    pred_d = nc.dram_tensor("pred", [BS, C, H, W], F32, kind=img_kind)
    targ_d = nc.dram_tensor("targ", [BS, C, H, W], F32, kind=img_kind)
    lr_d = nc.dram_tensor("lr", [BS, C, TH, TW], F32, kind=img_kind)
    mT_d = nc.dram_tensor("mT", [H, TH], F32, kind="ExternalInput")  # M^T
    # raw per-partition accumulator columns; host does the final sums.
    # cols [0, IMGS) = pix, [IMGS, 2*IMGS) = lr
    out_d = nc.dram_tensor("acc_out", [128, 2 * IMGS], F32, kind="ExternalOutput")

    if o["flat_img"]:
        # partition p holds rows 4p..4p+3 of the image: fully contiguous 8KB
        pred_v = pred_d.ap().rearrange("b c (p r) w -> b c p r w", p=128)
        targ_v = targ_d.ap().rearrange("b c (p r) w -> b c p r w", p=128)
        # stage-1 matmul contracts h = 4p + r
        mT1_v = mT_d.ap().rearrange("(p r) o -> p r o", p=128)
    else:
        # partition p holds row hc*128 + p: [b, c, p(h in chunk), hc, w]
        pred_v = pred_d.ap().rearrange("b c (hc p) w -> b c p hc w", p=128)
        targ_v = targ_d.ap().rearrange("b c (hc p) w -> b c p hc w", p=128)
        # stage-1 matmul contracts h = hc*128 + p
        mT1_v = mT_d.ap().rearrange("(hc p) o -> p hc o", p=128)
    # per-batch view: [b, p, c*hc, w]
    pred_bv = pred_d.ap().rearrange("b c (hc p) w -> b p (c hc) w", p=128)
    targ_bv = targ_d.ap().rearrange("b c (hc p) w -> b p (c hc) w", p=128)
    lr_v = lr_d.ap().rearrange("b c p w -> b p c w")          # [b, 128, C, 128]
    mT2_v = mT_d.ap().rearrange("(wc p) o -> p wc o", p=128)  # [128, WC, 128]

    targ_dma = nc.scalar if o["split_rings"] else nc.sync
    lr_dma = nc.gpsimd if o["lr_swdge"] else nc.sync

    with tile.TileContext(nc) as tc, ExitStack() as ctx:
        consts = ctx.enter_context(tc.tile_pool(name="consts", bufs=1))
        accs = ctx.enter_context(tc.tile_pool(name="accs", bufs=1))
        nbufs = 2 if o["batch_dma"] else o["bufs"]
        pred_p = ctx.enter_context(tc.tile_pool(name="pred_p", bufs=nbufs))
        targ_p = ctx.enter_context(tc.tile_pool(name="targ_p", bufs=nbufs))
        d_p = ctx.enter_context(tc.tile_pool(name="d_p", bufs=3))
        absd_p = None
        if not o["inplace_abs"]:
            absd_p = ctx.enter_context(tc.tile_pool(name="absd_p", bufs=3))
        t1t_p = ctx.enter_context(tc.tile_pool(name="t1t_p", bufs=3))
        lr_p = ctx.enter_context(tc.tile_pool(name="lr_p", bufs=2))
        small_p = ctx.enter_context(tc.tile_pool(name="small_p", bufs=3))
        ps_t1t = ctx.enter_context(tc.tile_pool(name="ps_t1t", bufs=2, space="PSUM"))
        ps_out2 = ctx.enter_context(tc.tile_pool(name="ps_out2", bufs=2, space="PSUM"))

        mT1_t = consts.tile([128, HC, TH], F32)
        nc.sync.dma_start(mT1_t[:], mT1_v)
        mT2_t = consts.tile([128, WC, TH], F32)
        nc.sync.dma_start(mT2_t[:], mT2_v)

        acc = accs.tile([128, 2 * IMGS], F32)
        pix_acc = acc[:, 0:IMGS]
        lr_acc = acc[:, IMGS:2 * IMGS]

        def image_compute(i, pred_im, targ_im, lr_col):
            """pred_im/targ_im: [128, HC, W] SBUF views; lr_col: [128, TW] view."""
            # pix partial: sum |pred - targ| per partition, one column per image
            d_t = d_p.tile([128, HC, W], F32, tag="d")
            nc.vector.tensor_sub(d_t[:], pred_im, targ_im)
            if o["inplace_abs"]:
                abs_out = d_t[:]
            else:
                absd_t = absd_p.tile([128, HC, W], F32, tag="absd")
                abs_out = absd_t[:]
            nc.scalar.activation(abs_out, d_t[:], AF.Abs,
                                 accum_out=pix_acc[:, i:i + 1])

            # T1^T[w, o] = (img^T @ Mh^T)[w, o], chunked over w.
            # Contraction over h is split as (k-index, partition) pairs; the
            # mT1 layout matches pred_im's h mapping in both modes.
            t1t_ps = ps_t1t.tile([128, WC, TH], F32, tag="t1t_ps")
            for wc in range(WC):
                for k in range(HC):
                    nc.tensor.matmul(
                        t1t_ps[:, wc, :],
                        lhsT=pred_im[:, k, bass.ts(wc, 128)],
                        rhs=mT1_t[:, k, :],
                        start=(k == 0), stop=(k == HC - 1),
                    )
            t1t_t = t1t_p.tile([128, WC, TH], F32, tag="t1t")
            nc.scalar.copy(t1t_t[:], t1t_ps[:])

            # out2[o, p] = sum_w T1^T[w, o] * Mw^T[w, p]
            out2_ps = ps_out2.tile([128, TW], F32, tag="out2")
            for wc in range(WC):
                nc.tensor.matmul(
                    out2_ps[:],
                    lhsT=t1t_t[:, wc, :],
                    rhs=mT2_t[:, wc, :],
                    start=(wc == 0), stop=(wc == WC - 1),
                )

            d2_t = small_p.tile([128, TW], F32, tag="d2")
            nc.vector.tensor_sub(d2_t[:], out2_ps[:], lr_col)
            absd2_t = small_p.tile([128, TW], F32, tag="absd2")
            nc.scalar.activation(absd2_t[:], d2_t[:], AF.Abs,
                                 accum_out=lr_acc[:, i:i + 1])

        for _rep in range(o["reps"]):
            for b in range(BS):
                lr_t = lr_p.tile([128, C, TW], F32, tag="lr")
                lr_dma.dma_start(lr_t[:], lr_v[b])
                if o["batch_dma"]:
                    pred_t = pred_p.tile([128, C * HC, W], F32, tag="pred")
                    nc.sync.dma_start(pred_t[:], pred_bv[b])
                    targ_t = targ_p.tile([128, C * HC, W], F32, tag="targ")
                    targ_dma.dma_start(targ_t[:], targ_bv[b])
                    if not o["compute"]:
                        continue
                    for c in range(C):
                        image_compute(b * C + c,
                                      pred_t[:, c * HC:(c + 1) * HC, :],
                                      targ_t[:, c * HC:(c + 1) * HC, :],
                                      lr_t[:, c, :])
                else:
                    for c in range(C):
                        pred_t = pred_p.tile([128, HC, W], F32, tag="pred")
                        nc.sync.dma_start(pred_t[:], pred_v[b, c])
                        targ_t = targ_p.tile([128, HC, W], F32, tag="targ")
                        targ_dma.dma_start(targ_t[:], targ_v[b, c])
                        if not o["compute"]:
                            continue
                        image_compute(b * C + c, pred_t[:], targ_t[:],
                                      lr_t[:, c, :])

        # ship the raw accumulator columns; host reduces in float64
        if not o["compute"]:
            nc.vector.memset(acc[:], 0.0)
        nc.sync.dma_start(out_d.ap(), acc[:])

    nc.compile()
    _CACHE[key] = nc
    return nc


def _make_runner(nc):
    """Build the sharded PJRT callable once (mirrors bass2jax.run_bass_via_pjrt,
    but reusable across calls so repeat invocations skip retrace/NEFF reload)."""
    import jax
    from jax.sharding import Mesh, PartitionSpec
    from jax.experimental.shard_map import shard_map
    from concourse.bass2jax import (_bass_exec_p, install_neuronx_cc_hook,
                                    partition_id_tensor)

    install_neuronx_cc_hook()
    in_names, out_names, out_avals = [], [], []
    for alloc in nc.m.functions[0].allocations:
        if not isinstance(alloc, mybir.MemoryLocationSet):
            continue
        name = alloc.memorylocations[0].name
        if alloc.kind == "ExternalInput":
            if nc.partition_id_tensor is None or name != nc.partition_id_tensor.name:
                in_names.append(name)
        elif alloc.kind == "ExternalOutput":
            out_names.append(name)
            out_avals.append(jax.core.ShapedArray(
                tuple(alloc.tensor_shape), mybir.dt.np(alloc.dtype)))
    n_params = len(in_names)
    n_outs = len(out_avals)
    all_names = list(in_names) + out_names
    if nc.partition_id_tensor is not None:
        all_names.append(nc.partition_id_tensor.name)

    def _body(*args):
        operands = list(args)
        if nc.partition_id_tensor is not None:
            operands.append(partition_id_tensor())
        return tuple(_bass_exec_p.bind(
            *operands,
            out_avals=tuple(out_avals),
            in_names=tuple(all_names),
            out_names=tuple(out_names),
            lowering_input_output_aliases=(),
            sim_require_finite=True,
            sim_require_nnan=True,
            nc=nc,
        ))

    devices = jax.devices()[:N_CORES]
    mesh = Mesh(np.asarray(devices), ("core",))
    in_specs = (PartitionSpec("core"),) * (n_params + n_outs)
    out_specs = (PartitionSpec("core"),) * n_outs
    sharded = jax.jit(
        shard_map(_body, mesh=mesh, in_specs=in_specs, out_specs=out_specs,
                  check_rep=False),
        keep_unused=True,
    )

    def run_concat(concat_by_name):
        """concat_by_name: input name -> global array (cores stacked on axis 0)."""
        concat_in = [np.ascontiguousarray(concat_by_name[name]) for name in in_names]
        zeros = [np.zeros((N_CORES * a.shape[0], *a.shape[1:]), a.dtype)
                 for a in out_avals]
        out_arrs = sharded(*concat_in, *zeros)
        return [
            {name: np.asarray(out_arrs[i]).reshape(N_CORES, *out_avals[i].shape)[c]
             for i, name in enumerate(out_names)}
            for c in range(N_CORES)
        ]

    def run(in_maps):
        return run_concat({
            name: np.concatenate([np.asarray(m[name]) for m in in_maps], axis=0)
            for name in in_names
        })

    run.run_concat = run_concat
    return run


def _runner():
    if "runner" not in _CACHE:
        _CACHE["runner"] = _make_runner(_build())
    return _CACHE["runner"]


def _in_maps(pred_hr, target_hr, lr_ref):
    mT = np.ascontiguousarray(_resize_matrix(H, TH).T)  # [512, 128]
    maps = []
    for i in range(N_CORES):
        sl = slice(i * BS, (i + 1) * BS)
        maps.append({
            "pred": np.ascontiguousarray(pred_hr[sl], dtype=np.float32),
            "targ": np.ascontiguousarray(target_hr[sl], dtype=np.float32),
            "lr": np.ascontiguousarray(lr_ref[sl], dtype=np.float32),
            "mT": mT,
        })
    return maps


def kernel(pred_hr, target_hr, lr_ref, scale):
    assert pred_hr.shape == (B, C, H, W) and target_hr.shape == (B, C, H, W)
    assert lr_ref.shape == (B, C, TH, TW)
    # the batch axis is the shard axis, so the full arrays already ARE the
    # per-core shards stacked along axis 0
    mT = np.ascontiguousarray(_resize_matrix(H, TH).T)  # [512, 128]
    try:
        results = _runner().run_concat({
            "pred": np.asarray(pred_hr, dtype=np.float32),
            "targ": np.asarray(target_hr, dtype=np.float32),
            "lr": np.asarray(lr_ref, dtype=np.float32),
            "mT": np.concatenate([mT] * N_CORES, axis=0),
        })
    except Exception:
        # fallback: the stock (uncached) dispatch path
        _CACHE.pop("runner", None)
        res = run_bass_kernel_spmd(_build(), _in_maps(pred_hr, target_hr, lr_ref),
                                   list(range(N_CORES)))
        results = res.results

    pix_sum = 0.0
    lr_sum = 0.0
    for i in range(N_CORES):
        a = results[i]["acc_out"].astype(np.float64)
        pix_sum += a[:, :IMGS].sum()
        lr_sum += a[:, IMGS:].sum()

    pix = np.float32(pix_sum / (B * C * H * W))
    lr_term = np.float32(lr_sum / (B * C * TH * TW))
    pair_term = np.float32(0.0)
    consist = np.float32(LAM_LR * lr_term + pair_term)
    total = np.float32(pix + LAM_CONSIST * consist)
    return (total, pix, consist, lr_term, pair_term)
